# revision 1
# baseline (speedup 1.0000x reference)
"""Causal self-attention (B=4, T=2048, C=1024, H=16, D=64) on 8 TRN2 NeuronCores.

Sharding: data-parallel over batch (4) x tensor-parallel over heads (2 groups
of 8 heads).  Core c handles batch c//2 and heads (c%2)*8 .. (c%2)*8+8.
Each core computes its QKV projection shard, causal flash-style attention for
its 8 heads, and a partial output projection (row-parallel).  The host sums
the two partials per batch and adds b_proj.

Device layouts (per core):
  xT   [1024, 2048]  x[b].T (channels on partitions), bf16
  wqk  [1025, 1024]  [Wq | Wk] columns for this head group; row 1024 = bias
  wv   [1025,  512]  Wv columns; row 1024 = bias
  wp   [ 512, 1024]  w_proj rows for this head group
  masks[ 128, 2048]  4 diagonal causal masks (kv-local x query-local)
  outT [1024, 2048]  partial (attn @ wp).T, before b_proj, fp32

All matmuls run in bf16 with fp32 PSUM accumulation.  Attention scores are
computed transposed (kv on partitions, queries free) so the softmax
probabilities feed the AV matmul directly as the moving operand; the
denominator comes for free from a ones column appended to V.  All pools live
in one scope so the Tile scheduler overlaps QKV, attention and projection.
"""

import os
import sys

for _p in (
    "/root/.axon_site",
    "/root/.axon_site/_ro/trn_rl_repo",
    "/root/.axon_site/_ro/pypackages",
    "/opt/trn_rl_repo",
):
    if os.path.isdir(_p) and _p not in sys.path:
        sys.path.append(_p)

from contextlib import ExitStack

import ml_dtypes
import numpy as np

import concourse.tile as tile
from concourse import bacc, mybir
from concourse.bass import ds, ts
from concourse.bass_utils import run_bass_kernel_spmd

F32 = mybir.dt.float32
F32R = mybir.dt.float32r
BF16 = mybir.dt.bfloat16
EXP = mybir.ActivationFunctionType.Exp
MULT = mybir.AluOpType.mult

B, T, C, H, D = 4, 2048, 1024, 16, 64
HPC = 8            # heads per core
CS = HPC * D       # 512 sharded channels
NC_ = C // 128     # 8 channel tiles
TB = T // 128      # 16 token blocks
TCH = T // 512     # 4 query chunks
SCALE = 1.0 / np.sqrt(D)

_CACHE = {}


def _build_program():
    nc = bacc.Bacc("TRN2", target_bir_lowering=False, debug=False)

    xT_d = nc.dram_tensor("xT", [C, T], BF16, kind="ExternalInput")
    wqk_d = nc.dram_tensor("wqk", [C + 1, 2 * CS], BF16, kind="ExternalInput")
    wv_d = nc.dram_tensor("wv", [C + 1, CS], BF16, kind="ExternalInput")
    wp_d = nc.dram_tensor("wp", [CS, C], BF16, kind="ExternalInput")
    mk_d = nc.dram_tensor("masks", [128, 2048], BF16, kind="ExternalInput")
    on_d = nc.dram_tensor("ones", [128, 512], BF16, kind="ExternalInput")
    outT_d = nc.dram_tensor("outT", [C, T], F32, kind="ExternalOutput")

    with tile.TileContext(nc) as tc, ExitStack() as ctx, \
            nc.allow_low_precision(reason="bf16 matmuls, f32r epilogue"):
        pool_c = ctx.enter_context(tc.tile_pool(name="const", bufs=1))
        pool_qk = ctx.enter_context(tc.tile_pool(name="qkT", bufs=1))
        pool_ve = ctx.enter_context(tc.tile_pool(name="vext", bufs=1))
        pool_x = ctx.enter_context(tc.tile_pool(name="xT", bufs=1))
        pool_w = ctx.enter_context(tc.tile_pool(name="wqk", bufs=2))
        pool_wv = ctx.enter_context(tc.tile_pool(name="wv", bufs=1))
        pool_p = ctx.enter_context(tc.tile_pool(name="P", bufs=4))
        pool_r = ctx.enter_context(tc.tile_pool(name="rec", bufs=3))
        pool_tmp = ctx.enter_context(tc.tile_pool(name="psb", bufs=14))
        pool_y = ctx.enter_context(tc.tile_pool(name="yT", bufs=1))
        pool_wp = ctx.enter_context(tc.tile_pool(name="wp", bufs=1))
        pool_o = ctx.enter_context(tc.tile_pool(name="out", bufs=3))
        ps_mm = ctx.enter_context(tc.tile_pool(name="psmm", bufs=3, space="PSUM"))
        ps_y = ctx.enter_context(tc.tile_pool(name="psy", bufs=2, space="PSUM"))

        ones = pool_c.tile([128, 512], BF16)
        nc.sync.dma_start(ones[:], on_d.ap()[:])

        # first m-tile's weights before the big xT load so matmuls start early
        w0_sb = pool_w.tile([128, NC_, 128], BF16, tag="wqk", name="w0_sb")
        nc.sync.dma_start(
            w0_sb[:],
            wqk_d.ap()[0:C, ts(0, 128)].rearrange("(c p) f -> p c f", p=128),
        )
        bq0_sb = pool_w.tile([1, 128], BF16, tag="bqk", name="bq0_sb")
        nc.sync.dma_start(bq0_sb[:], wqk_d.ap()[C : C + 1, ts(0, 128)])

        # xT split per channel tile: QKV matmuls depend only on their slice
        xT = pool_x.tile([128, NC_, T], BF16)
        xT_ap = xT_d.ap().rearrange("(c p) t -> p c t", p=128)
        for ci in range(NC_):
            nc.sync.dma_start(xT[:, ci], xT_ap[:, ci])

        wv_sb = pool_wv.tile([128, NC_, CS], BF16)
        nc.sync.dma_start(
            wv_sb[:], wv_d.ap()[0:C].rearrange("(c p) f -> p c f", p=128)
        )
        bv_sb = pool_wv.tile([1, CS], BF16)
        nc.sync.dma_start(bv_sb[:], wv_d.ap()[C : C + 1])

        # qkT[p, m, t]: feature-major q|k activations, feature f = m*128+p
        qkT = pool_qk.tile([128, 2 * CS // 128, T], BF16)
        # v_ext[p, tb, h*65+d]; col h*65+64 holds ones (the denominator trick)
        vext = pool_ve.tile([128, TB, HPC * 65], BF16)
        nc.sync.dma_start(
            vext[:].rearrange("p tb (h s) -> p tb h s", s=65)[:, :, :, 64],
            on_d.ap()[:, 0 : TB * HPC].rearrange("p (tb h) -> p tb h", h=HPC),
        )
        masks = pool_c.tile([128, 4, 512], BF16)
        nc.sync.dma_start(masks[:], mk_d.ap().rearrange("p (r i) -> p r i", r=4))
        yT = pool_y.tile([128, CS // 128, T], BF16)

        # ---------------- QKV projection ----------------
        def qk_mtile(m, w_sb=None, bq_sb=None):
            if w_sb is None:
                w_sb = pool_w.tile([128, NC_, 128], BF16, tag="wqk")
                nc.sync.dma_start(
                    w_sb[:],
                    wqk_d.ap()[0:C, ts(m, 128)].rearrange("(c p) f -> p c f",
                                                          p=128),
                )
                bq_sb = pool_w.tile([1, 128], BF16, tag="bqk")
                nc.sync.dma_start(bq_sb[:], wqk_d.ap()[C : C + 1, ts(m, 128)])
            for tch in range(TCH):
                ps = ps_mm.tile([128, 1024], F32, tag="mm", name="qkps")
                for ci in range(NC_):
                    nc.tensor.matmul(
                        ps[:, 0:512], w_sb[:, ci], xT[:, ci, ts(tch, 512)],
                        start=(ci == 0), stop=False,
                    )
                nc.tensor.matmul(
                    ps[:, 0:512], bq_sb[:], ones[0:1, :], start=False, stop=True
                )
                nc.any.tensor_copy(out=qkT[:, m, ts(tch, 512)], in_=ps[:, 0:512])

        def v_phase():
            for tb in range(TB):
                ps = ps_mm.tile([128, 1024], F32, tag="mm", name="vps")
                for ci in range(NC_):
                    nc.tensor.matmul(
                        ps[:, 0:512], xT[:, ci, ts(tb, 128)], wv_sb[:, ci],
                        start=(ci == 0), stop=False,
                    )
                nc.tensor.matmul(
                    ps[:, 0:512], ones[0:1, 0:128], bv_sb[:],
                    start=False, stop=True,
                )
                nc.any.tensor_copy(
                    out=vext[:, tb].rearrange("p (h s) -> p h s", s=65)[:, :, 0:64],
                    in_=ps[:, 0:512].rearrange("p (h d) -> p h d", d=64),
                )

        # ---------------- attention for one head pair ----------------
        def attn_pair(p):
            den2 = pool_r.tile([32, 128], F32R, tag="den2")
            psbs = {}
            for I in range(TCH):
                nj = 4 * I + 4
                psy = [
                    ps_y.tile([65, 512], F32, tag="psy", name=f"psy{hb_}")
                    for hb_ in range(2)
                ]
                for jp in range(nj // 2):
                    pss = [
                        ps_mm.tile([128, 1024], F32, tag="mm", name=f"pss{hb_}")
                        for hb_ in range(2)
                    ]
                    for hb in range(2):
                        base = hb * 64
                        for jj in range(2):
                            j = 2 * jp + jj
                            nc.tensor.matmul(
                                pss[hb][:, ts(jj, 512)],
                                qkT[base : base + 64, 4 + p, ts(j, 128)],
                                qkT[base : base + 64, p, ds(I * 512, 512)],
                            )
                    P = [
                        pool_p.tile([128, 1024], BF16, tag="P", name=f"P{hb_}")
                        for hb_ in range(2)
                    ]
                    for hb in range(2):
                        nc.scalar.activation(
                            P[hb][:], pss[hb][:], EXP, scale=float(SCALE)
                        )
                    for jj in range(2):
                        r = 2 * jp + jj - 4 * I
                        if r >= 0:
                            for hb in range(2):
                                nc.vector.tensor_tensor(
                                    P[hb][:, ts(jj, 512)],
                                    P[hb][:, ts(jj, 512)],
                                    masks[:, r],
                                    MULT,
                                )
                    for jj in range(2):
                        j = 2 * jp + jj
                        for hb in range(2):
                            h = 2 * p + hb
                            nc.tensor.matmul(
                                psy[hb][:],
                                vext[:, j, ds(h * 65, 65)],
                                P[hb][:, ts(jj, 512)],
                                start=(j == 0),
                                stop=(j == nj - 1),
                            )
                # free the AV psum banks fast: copy to SBUF (kept until pair
                # end) and scatter the denominator row into den2 (4 rows x
                # 128) for one batched reciprocal.
                for hb in range(2):
                    psb = pool_tmp.tile(
                        [65, 512], F32R, tag="psysb", name=f"psb{hb}"
                    )
                    nc.vector.tensor_copy(out=psb[:], in_=psy[hb][:])
                    g = (I * 2 + hb) * 4
                    nc.sync.dma_start(den2[g : g + 4, :], psb[64:65, :])
                    psbs[(I, hb)] = psb
            # pair-end epilogue: one reciprocal for all 8 denominator rows,
            # gpsimd-broadcast each recip row, normalize on lanes 0..63, DMA
            # into yT.  Overlaps the next pair's attention.
            rec2 = pool_r.tile([32, 128], F32R, tag="rec2")
            nc.vector.reciprocal(rec2[:], den2[:])
            for I in range(TCH):
                for hb in range(2):
                    g = (I * 2 + hb) * 4
                    rec0 = pool_r.tile([1, 512], F32R, tag="rec0")
                    nc.sync.dma_start(rec0[:], rec2[g : g + 4, :])
                    bc = pool_r.tile([64, 512], F32R, tag="bc")
                    nc.gpsimd.partition_broadcast(bc[:], rec0[:])
                    yn = pool_tmp.tile([64, 512], BF16, tag="yn")
                    nc.vector.tensor_tensor(
                        yn[:], psbs[(I, hb)][0:64, :], bc[:], MULT
                    )
                    nc.sync.dma_start(
                        yT[hb * 64 : hb * 64 + 64, p, ds(I * 512, 512)], yn[:]
                    )

        # ---------------- output projection (one m-tile) ----------------
        wp_sb = pool_wp.tile([128, CS // 128, C], BF16)
        outT_ap = outT_d.ap().rearrange("(co p) t -> p co t", p=128)

        def proj_co(co):
            for tch in range(TCH):
                ps = ps_mm.tile([128, 1024], F32, tag="mm", name="projps")
                for cit in range(CS // 128):
                    nc.tensor.matmul(
                        ps[:, 0:512],
                        wp_sb[:, cit, ts(co, 128)],
                        yT[:, cit, ts(tch, 512)],
                        start=(cit == 0),
                        stop=(cit == CS // 128 - 1),
                    )
                ot = pool_o.tile([128, 512], F32, tag="out")
                nc.any.tensor_copy(out=ot[:], in_=ps[:, 0:512])
                nc.sync.dma_start(outT_ap[:, co, ts(tch, 512)], ot[:])

        # emission order = scheduling priority: QKV for pair 0 first, then
        # interleave remaining QKV m-tiles with attention pairs so ACT/DVE
        # softmax work overlaps the PE-heavy projection phases.
        qk_mtile(0, w0_sb, bq0_sb)
        qk_mtile(4)
        v_phase()
        attn_pair(0)
        qk_mtile(1)
        qk_mtile(5)
        attn_pair(1)
        qk_mtile(2)
        qk_mtile(6)
        attn_pair(2)
        qk_mtile(3)
        qk_mtile(7)
        nc.sync.dma_start(
            wp_sb[:], wp_d.ap().rearrange("(c p) f -> p c f", p=128)
        )
        attn_pair(3)
        for co in range(C // 128):
            proj_co(co)

    nc.compile()
    return nc


def _masks_host() -> np.ndarray:
    # masks[p, r*512 + i] = 1.0 if i >= r*128 + p else 0.0
    p = np.arange(128)[:, None]
    i = np.arange(512)[None, :]
    out = np.empty((128, 4, 512), dtype=np.float32)
    for r in range(4):
        out[:, r, :] = (i >= r * 128 + p).astype(np.float32)
    return out.reshape(128, 2048)


def kernel(x, w_qkv, b_qkv, w_proj, b_proj):
    x = np.asarray(x, dtype=np.float32)
    w_qkv = np.asarray(w_qkv, dtype=np.float32)
    b_qkv = np.asarray(b_qkv, dtype=np.float32)
    w_proj = np.asarray(w_proj, dtype=np.float32)
    b_proj = np.asarray(b_proj, dtype=np.float32)

    if "nc" not in _CACHE:
        _CACHE["nc"] = _build_program()
    nc = _CACHE["nc"]

    bf = ml_dtypes.bfloat16
    masks = _masks_host().astype(bf)
    ones = np.ones((128, 512), dtype=bf)

    in_maps = []
    for c in range(8):
        b, hg = c // 2, c % 2
        sl = slice(hg * CS, (hg + 1) * CS)
        wq = np.concatenate(
            [w_qkv[:, sl], w_qkv[:, C + hg * CS : C + (hg + 1) * CS]], axis=1
        )
        bq = np.concatenate([b_qkv[sl], b_qkv[C + hg * CS : C + (hg + 1) * CS]])
        wv = w_qkv[:, 2 * C + hg * CS : 2 * C + (hg + 1) * CS]
        bv = b_qkv[2 * C + hg * CS : 2 * C + (hg + 1) * CS]
        in_maps.append({
            "xT": np.ascontiguousarray(x[b].T).astype(bf),
            "wqk": np.concatenate([wq, bq[None, :]], axis=0).astype(bf),
            "wv": np.concatenate([wv, bv[None, :]], axis=0).astype(bf),
            "wp": np.ascontiguousarray(w_proj[hg * CS : (hg + 1) * CS]).astype(bf),
            "masks": masks,
            "ones": ones,
        })

    _CACHE["in_maps"] = in_maps
    res = run_bass_kernel_spmd(nc, in_maps, core_ids=list(range(8)))

    out = np.empty((B, T, C), dtype=np.float32)
    for b in range(B):
        out[b] = res.results[2 * b]["outT"].T
        out[b] += res.results[2 * b + 1]["outT"].T
        out[b] += b_proj
    return out



# revision 4
# speedup vs baseline: 1.2439x; 1.2439x over previous
"""Causal self-attention (B=4, T=2048, C=1024, H=16, D=64) on 8 TRN2 NeuronCores.

Sharding: data-parallel over batch (4) x tensor-parallel over heads (2 groups
of 8 heads).  Core c handles batch c//2 and heads (c%2)*8 .. (c%2)*8+8.
Each core computes its QKV projection shard, causal flash-style attention for
its 8 heads, and a partial output projection (row-parallel).  The host sums
the two partials per batch and adds b_proj.

Device layouts (per core):
  xT   [1024, 2048]  x[b].T (channels on partitions), bf16
  wqk  [1024, 1024]  [Wq | Wk] columns for this head group, bf16
  bqT  [ 128,    8]  qk bias, feature-tile-major (bqT[p,m] = b[m*128+p])
  wv   [1025,  512]  Wv columns; row 1024 = bias
  wp   [ 512, 1024]  w_proj rows for this head group
  masks[ 128, 2048]  4 diagonal causal masks (kv-local x query-local)
  outT [1024, 2048]  partial (attn @ wp).T, before b_proj, fp32

All matmuls run in bf16 with fp32 PSUM accumulation.  Attention scores are
computed transposed (kv on partitions, queries free) so the softmax
probabilities feed the AV matmul directly as the moving operand; the
denominator comes for free from a ones column appended to V.  Scores/AV/mask
are truncated to the causal band at 128-column granularity.  The two head
halves of each pair run as concurrent row-tiled (K=64) matmuls.  Copies off
PSUM carry the qk bias (per-partition) and are split ACT/DVE so ACT does
almost nothing but exp.  Warmup matmuls keep the PE HAM-warm during the
initial x DMA.
"""

import os
import sys

for _p in (
    "/root/.axon_site",
    "/root/.axon_site/_ro/trn_rl_repo",
    "/root/.axon_site/_ro/pypackages",
    "/opt/trn_rl_repo",
):
    if os.path.isdir(_p) and _p not in sys.path:
        sys.path.append(_p)

from contextlib import ExitStack

import ml_dtypes
import numpy as np

import concourse.tile as tile
from concourse import bacc, mybir
from concourse.bass import ds, ts
from concourse.bass_utils import run_bass_kernel_spmd

F32 = mybir.dt.float32
F32R = mybir.dt.float32r
BF16 = mybir.dt.bfloat16
EXP = mybir.ActivationFunctionType.Exp
IDENT = mybir.ActivationFunctionType.Identity
MULT = mybir.AluOpType.mult
ADD = mybir.AluOpType.add

B, T, C, H, D = 4, 2048, 1024, 16, 64
HPC = 8            # heads per core
CS = HPC * D       # 512 sharded channels
NC_ = C // 128     # 8 channel tiles
TB = T // 128      # 16 token blocks
TCH = T // 512     # 4 query chunks
SCALE = 1.0 / np.sqrt(D)

_CACHE = {}


def _build_program():
    nc = bacc.Bacc("TRN2", target_bir_lowering=False, debug=False)

    xT_d = nc.dram_tensor("xT", [C, T], BF16, kind="ExternalInput")
    wqk_d = nc.dram_tensor("wqk", [C, 2 * CS], BF16, kind="ExternalInput")
    bqT_d = nc.dram_tensor("bqT", [128, NC_], F32, kind="ExternalInput")
    wv_d = nc.dram_tensor("wv", [C + 1, CS], BF16, kind="ExternalInput")
    wp_d = nc.dram_tensor("wp", [CS, C], BF16, kind="ExternalInput")
    mk_d = nc.dram_tensor("masks", [128, 2048], BF16, kind="ExternalInput")
    on_d = nc.dram_tensor("ones", [128, 512], BF16, kind="ExternalInput")
    outT_d = nc.dram_tensor("outT", [C, T], F32, kind="ExternalOutput")

    with tile.TileContext(nc) as tc, ExitStack() as ctx, \
            nc.allow_low_precision(reason="bf16 matmuls, f32r epilogue"):
        pool_c = ctx.enter_context(tc.tile_pool(name="const", bufs=1))
        pool_qk = ctx.enter_context(tc.tile_pool(name="qkT", bufs=1))
        pool_ve = ctx.enter_context(tc.tile_pool(name="vext", bufs=1))
        pool_x = ctx.enter_context(tc.tile_pool(name="xT", bufs=1))
        pool_w = ctx.enter_context(tc.tile_pool(name="wqk", bufs=1))
        pool_wv = ctx.enter_context(tc.tile_pool(name="wv", bufs=1))
        pool_p = ctx.enter_context(tc.tile_pool(name="P", bufs=4))
        pool_r = ctx.enter_context(tc.tile_pool(name="rec", bufs=3))
        pool_tmp = ctx.enter_context(tc.tile_pool(name="psb", bufs=6))
        pool_y = ctx.enter_context(tc.tile_pool(name="yT", bufs=1))
        pool_wp = ctx.enter_context(tc.tile_pool(name="wp", bufs=1))
        pool_o = ctx.enter_context(tc.tile_pool(name="out", bufs=3))
        ps_a = ctx.enter_context(tc.tile_pool(name="psa", bufs=2, space="PSUM"))
        ps_s = ctx.enter_context(tc.tile_pool(name="pss", bufs=2, space="PSUM"))
        ps_y = ctx.enter_context(tc.tile_pool(name="psy", bufs=2, space="PSUM"))

        ones = pool_c.tile([128, 512], BF16)
        nc.sync.dma_start(ones[:], on_d.ap()[:])

        # first m-tile's weights + bias before the big xT load
        w0_sb = pool_w.tile([128, NC_, 128], BF16, tag="w0")
        nc.sync.dma_start(
            w0_sb[:],
            wqk_d.ap()[0:C, 0:128].rearrange("(c p) f -> p c f", p=128),
        )
        bqT_sb = pool_c.tile([128, NC_], F32, tag="bqT")
        nc.sync.dma_start(bqT_sb[:], bqT_d.ap()[:])
        masks = pool_c.tile([128, 4, 512], BF16, tag="mk")
        nc.sync.dma_start(masks[:], mk_d.ap().rearrange("p (r i) -> p r i", r=4))

        # warmup matmuls: keep the PE busy (HAM warm) while xT streams in
        wu_ps = ps_a.tile([128, 512], F32, tag="mm", name="warm")
        for i in range(48):
            nc.tensor.matmul(
                wu_ps[:], ones[:, 0:128], ones[:], start=(i == 0), stop=(i == 47)
            )

        # xT split per channel tile: QKV matmuls depend only on their slice
        xT = pool_x.tile([128, NC_, T], BF16)
        xT_ap = xT_d.ap().rearrange("(c p) t -> p c t", p=128)
        for ci in range(NC_):
            nc.sync.dma_start(xT[:, ci], xT_ap[:, ci])

        wv_sb = pool_wv.tile([128, NC_, CS], BF16)
        nc.sync.dma_start(
            wv_sb[:], wv_d.ap()[0:C].rearrange("(c p) f -> p c f", p=128)
        )
        bv_sb = pool_wv.tile([1, CS], BF16)
        nc.sync.dma_start(bv_sb[:], wv_d.ap()[C : C + 1])

        # remaining qk weight m-tiles (cols 128..1024) and wp, prefetched
        wr_sb = pool_w.tile([128, NC_, 7 * 128], BF16, tag="wr")
        nc.sync.dma_start(
            wr_sb[:],
            wqk_d.ap()[0:C, 128 : 8 * 128].rearrange("(c p) f -> p c f", p=128),
        )
        wp_sb = pool_wp.tile([128, CS // 128, C], BF16)
        nc.sync.dma_start(
            wp_sb[:], wp_d.ap().rearrange("(c p) f -> p c f", p=128)
        )

        # qkT[p, m, t]: feature-major q|k activations, feature f = m*128+p
        qkT = pool_qk.tile([128, 2 * CS // 128, T], BF16)
        # v_ext[p, tb, h*65+d]; col h*65+64 holds ones (the denominator trick)
        vext = pool_ve.tile([128, TB, HPC * 65], BF16)
        nc.sync.dma_start(
            vext[:].rearrange("p tb (h s) -> p tb h s", s=65)[:, :, :, 64],
            on_d.ap()[:, 0 : TB * HPC].rearrange("p (tb h) -> p tb h", h=HPC),
        )
        yT = pool_y.tile([128, CS // 128, T], BF16)

        # ---------------- QKV projection ----------------
        def qk_mtile(m, on_act):
            w = w0_sb[:, :, :] if m == 0 else wr_sb[:, :, ds((m - 1) * 128, 128)]
            for tch in range(TCH):
                ps = ps_a.tile([128, 512], F32, tag="mm", name="qkps")
                for ci in range(NC_):
                    nc.tensor.matmul(
                        ps[:], w[:, ci], xT[:, ci, ts(tch, 512)],
                        start=(ci == 0), stop=(ci == NC_ - 1),
                    )
                if on_act:
                    nc.scalar.activation(
                        qkT[:, m, ts(tch, 512)], ps[:], IDENT,
                        bias=bqT_sb[:, m : m + 1],
                    )
                else:
                    nc.vector.tensor_scalar(
                        qkT[:, m, ts(tch, 512)], ps[:],
                        bqT_sb[:, m : m + 1], None, ADD,
                    )

        def v_phase():
            for tb in range(TB):
                ps = ps_a.tile([128, 512], F32, tag="mm", name="vps")
                for ci in range(NC_):
                    nc.tensor.matmul(
                        ps[:], xT[:, ci, ts(tb, 128)], wv_sb[:, ci],
                        start=(ci == 0), stop=False,
                    )
                nc.tensor.matmul(
                    ps[:], ones[0:1, 0:128], bv_sb[:], start=False, stop=True
                )
                nc.scalar.copy(
                    vext[:, tb].rearrange("p (h s) -> p h s", s=65)[:, :, 0:64],
                    ps[:].rearrange("p (h d) -> p h d", d=64),
                )

        # ---------------- attention for one head pair ----------------
        def attn_pair(p):
            for I in range(TCH):
                nj = 4 * I + 4
                psy = [
                    ps_y.tile([65, 512], F32, tag="psy", name=f"psy{hb_}")
                    for hb_ in range(2)
                ]
                for jp in range(nj // 2):
                    pss = [
                        ps_s.tile([128, 1024], F32, tag="sc", name=f"pss{hb_}")
                        for hb_ in range(2)
                    ]
                    r0 = 2 * jp - 4 * I
                    for hb in range(2):
                        base = hb * 64
                        for jj in range(2):
                            j = 2 * jp + jj
                            o = max(j - 4 * I, 0) * 128
                            nc.tensor.matmul(
                                pss[hb][:, ds(jj * 512 + o, 512 - o)],
                                qkT[base : base + 64, 4 + p, ts(j, 128)],
                                qkT[base : base + 64, p, ds(I * 512 + o, 512 - o)],
                            )
                    st = max(r0, 0) * 128
                    P = [
                        pool_p.tile([128, 1024], BF16, tag="P", name=f"P{hb_}")
                        for hb_ in range(2)
                    ]
                    for hb in range(2):
                        nc.scalar.activation(
                            P[hb][:, ds(st, 1024 - st)],
                            pss[hb][:, ds(st, 1024 - st)], EXP,
                            scale=float(SCALE),
                        )
                    for jj in range(2):
                        r = 2 * jp + jj - 4 * I
                        if r >= 0:
                            o = 128 * r
                            for hb in range(2):
                                nc.vector.tensor_tensor(
                                    P[hb][:, ds(jj * 512 + o, 512 - o)],
                                    P[hb][:, ds(jj * 512 + o, 512 - o)],
                                    masks[:, r, ds(o, 512 - o)],
                                    MULT,
                                )
                    for jj in range(2):
                        j = 2 * jp + jj
                        o = max(j - 4 * I, 0) * 128
                        for hb in range(2):
                            h = 2 * p + hb
                            nc.tensor.matmul(
                                psy[hb][:, ds(o, 512 - o)],
                                vext[:, j, ds(h * 65, 65)],
                                P[hb][:, ds(jj * 512 + o, 512 - o)],
                                start=(j == 0),
                                stop=(j == nj - 1),
                            )
                # per-I epilogue: evacuate psy, reciprocal of the ones-row,
                # gpsimd-broadcast, normalize, DMA into yT.  Overlaps the
                # next chunk's attention; unblocks proj per-tch early.
                den = pool_r.tile([8, 128], F32R, tag="den")
                psbs = []
                for hb in range(2):
                    psb = pool_tmp.tile([65, 512], F32R, tag="psb")
                    nc.vector.tensor_copy(out=psb[:], in_=psy[hb][:])
                    nc.sync.dma_start(den[hb * 4 : hb * 4 + 4, :], psb[64:65, :])
                    psbs.append(psb)
                rec = pool_r.tile([8, 128], F32R, tag="rec")
                nc.vector.reciprocal(rec[:], den[:])
                for hb in range(2):
                    rec0 = pool_r.tile([1, 512], F32R, tag="rec0")
                    nc.sync.dma_start(rec0[:], rec[hb * 4 : hb * 4 + 4, :])
                    bc = pool_r.tile([64, 512], F32R, tag="bc")
                    nc.gpsimd.partition_broadcast(bc[:], rec0[:])
                    yn = pool_tmp.tile([64, 512], BF16, tag="yn")
                    nc.vector.tensor_tensor(yn[:], psbs[hb][0:64, :], bc[:], MULT)
                    nc.sync.dma_start(
                        yT[hb * 64 : hb * 64 + 64, p, ds(I * 512, 512)], yn[:]
                    )

        # ---------------- output projection (one token chunk) ----------------
        outT_ap = outT_d.ap().rearrange("(co p) t -> p co t", p=128)

        def proj_tch(tch):
            for co in range(C // 128):
                ps = ps_a.tile([128, 512], F32, tag="mm", name="projps")
                for cit in range(CS // 128):
                    nc.tensor.matmul(
                        ps[:],
                        wp_sb[:, cit, ts(co, 128)],
                        yT[:, cit, ts(tch, 512)],
                        start=(cit == 0),
                        stop=(cit == CS // 128 - 1),
                    )
                ot = pool_o.tile([128, 512], F32, tag="out")
                nc.scalar.copy(ot[:], ps[:])
                nc.sync.dma_start(outT_ap[:, co, ts(tch, 512)], ot[:])

        # emission order = scheduling priority: QKV for pair 0 first, then
        # interleave remaining QKV m-tiles with attention pairs so the PE
        # stays fed while ACT grinds through exp; proj per-tch at the end
        # starts as soon as pair 3 finishes each chunk.
        qk_mtile(0, on_act=True)
        qk_mtile(4, on_act=True)
        v_phase()
        attn_pair(0)
        qk_mtile(1, on_act=False)
        qk_mtile(5, on_act=False)
        attn_pair(1)
        qk_mtile(2, on_act=False)
        qk_mtile(6, on_act=False)
        attn_pair(2)
        qk_mtile(3, on_act=False)
        qk_mtile(7, on_act=False)
        attn_pair(3)
        for tch in range(TCH):
            proj_tch(tch)

    nc.compile()
    return nc


def _masks_host() -> np.ndarray:
    # masks[p, r*512 + i] = 1.0 if i >= r*128 + p else 0.0
    p = np.arange(128)[:, None]
    i = np.arange(512)[None, :]
    out = np.empty((128, 4, 512), dtype=np.float32)
    for r in range(4):
        out[:, r, :] = (i >= r * 128 + p).astype(np.float32)
    return out.reshape(128, 2048)


def kernel(x, w_qkv, b_qkv, w_proj, b_proj):
    x = np.asarray(x, dtype=np.float32)
    w_qkv = np.asarray(w_qkv, dtype=np.float32)
    b_qkv = np.asarray(b_qkv, dtype=np.float32)
    w_proj = np.asarray(w_proj, dtype=np.float32)
    b_proj = np.asarray(b_proj, dtype=np.float32)

    if "nc" not in _CACHE:
        _CACHE["nc"] = _build_program()
    nc = _CACHE["nc"]

    bf = ml_dtypes.bfloat16
    masks = _masks_host().astype(bf)
    ones = np.ones((128, 512), dtype=bf)

    in_maps = []
    for c in range(8):
        b, hg = c // 2, c % 2
        sl = slice(hg * CS, (hg + 1) * CS)
        wq = np.concatenate(
            [w_qkv[:, sl], w_qkv[:, C + hg * CS : C + (hg + 1) * CS]], axis=1
        )
        bq = np.concatenate([b_qkv[sl], b_qkv[C + hg * CS : C + (hg + 1) * CS]])
        wv = w_qkv[:, 2 * C + hg * CS : 2 * C + (hg + 1) * CS]
        bv = b_qkv[2 * C + hg * CS : 2 * C + (hg + 1) * CS]
        in_maps.append({
            "xT": np.ascontiguousarray(x[b].T).astype(bf),
            "wqk": np.ascontiguousarray(wq).astype(bf),
            "bqT": np.ascontiguousarray(bq.reshape(NC_, 128).T).astype(np.float32),
            "wv": np.concatenate([wv, bv[None, :]], axis=0).astype(bf),
            "wp": np.ascontiguousarray(w_proj[hg * CS : (hg + 1) * CS]).astype(bf),
            "masks": masks,
            "ones": ones,
        })

    _CACHE["in_maps"] = in_maps
    res = run_bass_kernel_spmd(nc, in_maps, core_ids=list(range(8)))

    out = np.empty((B, T, C), dtype=np.float32)
    for b in range(B):
        out[b] = res.results[2 * b]["outT"].T
        out[b] += res.results[2 * b + 1]["outT"].T
        out[b] += b_proj
    return out


# revision 5
# speedup vs baseline: 1.2863x; 1.0341x over previous
"""Causal self-attention (B=4, T=2048, C=1024, H=16, D=64) on 8 TRN2 NeuronCores.

Sharding: data-parallel over batch (4) x tensor-parallel over heads (2 groups
of 8 heads).  Core c handles batch c//2 and heads (c%2)*8 .. (c%2)*8+8.
Each core computes its QKV projection shard, causal flash-style attention for
its 8 heads, and a partial output projection (row-parallel).  The host sums
the two partials per batch and adds b_proj.

Device layouts (per core):
  xT   [1024, 2048]  x[b].T (channels on partitions), bf16
  wqk  [1024, 1024]  [Wq | Wk] columns for this head group, bf16
  bqT  [ 128,    8]  qk bias, feature-tile-major (bqT[p,m] = b[m*128+p])
  wv   [1025,  512]  Wv columns; row 1024 = bias
  wp   [ 512, 1024]  w_proj rows for this head group
  masks[ 128, 2048]  4 diagonal causal masks (kv-local x query-local)
  outT [1024, 2048]  partial (attn @ wp).T, before b_proj, fp32

All matmuls run in bf16 with fp32 PSUM accumulation.  Attention scores are
computed transposed (kv on partitions, queries free) so the softmax
probabilities feed the AV matmul directly as the moving operand; the
denominator comes for free from a ones column appended to V.  Scores/AV/mask
are truncated to the causal band at 128-column granularity.  The two head
halves of each pair run as concurrent row-tiled (K=64) matmuls.  Copies off
PSUM carry the qk bias (per-partition) and are split ACT/DVE so ACT does
almost nothing but exp.  Warmup matmuls keep the PE HAM-warm during the
initial x DMA.
"""

import os
import sys

for _p in (
    "/root/.axon_site",
    "/root/.axon_site/_ro/trn_rl_repo",
    "/root/.axon_site/_ro/pypackages",
    "/opt/trn_rl_repo",
):
    if os.path.isdir(_p) and _p not in sys.path:
        sys.path.append(_p)

from contextlib import ExitStack

import ml_dtypes
import numpy as np

import concourse.tile as tile
from concourse import bacc, mybir
from concourse.bass import ds, ts
from concourse.bass_utils import run_bass_kernel_spmd

F32 = mybir.dt.float32
F32R = mybir.dt.float32r
BF16 = mybir.dt.bfloat16
EXP = mybir.ActivationFunctionType.Exp
IDENT = mybir.ActivationFunctionType.Identity
MULT = mybir.AluOpType.mult
ADD = mybir.AluOpType.add

B, T, C, H, D = 4, 2048, 1024, 16, 64
HPC = 8            # heads per core
CS = HPC * D       # 512 sharded channels
NC_ = C // 128     # 8 channel tiles
TB = T // 128      # 16 token blocks
TCH = T // 512     # 4 query chunks
SCALE = 1.0 / np.sqrt(D)

_CACHE = {}


def _build_program():
    nc = bacc.Bacc("TRN2", target_bir_lowering=False, debug=False)

    xT_d = nc.dram_tensor("xT", [C, T], BF16, kind="ExternalInput")
    wqk_d = nc.dram_tensor("wqk", [C, 2 * CS], BF16, kind="ExternalInput")
    bqT_d = nc.dram_tensor("bqT", [128, NC_], F32, kind="ExternalInput")
    wv_d = nc.dram_tensor("wv", [C + 1, CS], BF16, kind="ExternalInput")
    wp_d = nc.dram_tensor("wp", [CS, C], BF16, kind="ExternalInput")
    mk_d = nc.dram_tensor("masks", [128, 2048], BF16, kind="ExternalInput")
    on_d = nc.dram_tensor("ones", [128, 512], BF16, kind="ExternalInput")
    outT_d = nc.dram_tensor("outT", [C, T], F32, kind="ExternalOutput")

    with tile.TileContext(nc) as tc, ExitStack() as ctx, \
            nc.allow_low_precision(reason="bf16 matmuls, f32r epilogue"):
        pool_c = ctx.enter_context(tc.tile_pool(name="const", bufs=1))
        pool_qk = ctx.enter_context(tc.tile_pool(name="qkT", bufs=1))
        pool_ve = ctx.enter_context(tc.tile_pool(name="vext", bufs=1))
        pool_x = ctx.enter_context(tc.tile_pool(name="xT", bufs=1))
        pool_w = ctx.enter_context(tc.tile_pool(name="wqk", bufs=1))
        pool_wv = ctx.enter_context(tc.tile_pool(name="wv", bufs=1))
        pool_p = ctx.enter_context(tc.tile_pool(name="P", bufs=4))
        pool_r = ctx.enter_context(tc.tile_pool(name="rec", bufs=3))
        pool_tmp = ctx.enter_context(tc.tile_pool(name="psb", bufs=6))
        pool_y = ctx.enter_context(tc.tile_pool(name="yT", bufs=1))
        pool_wp = ctx.enter_context(tc.tile_pool(name="wp", bufs=1))
        pool_o = ctx.enter_context(tc.tile_pool(name="out", bufs=3))
        ps_a = ctx.enter_context(tc.tile_pool(name="psa", bufs=2, space="PSUM"))
        ps_s = ctx.enter_context(tc.tile_pool(name="pss", bufs=2, space="PSUM"))
        ps_y = ctx.enter_context(tc.tile_pool(name="psy", bufs=2, space="PSUM"))

        ones = pool_c.tile([128, 512], BF16)
        nc.sync.dma_start(ones[:], on_d.ap()[:])

        # first m-tile's weights + bias before the big xT load
        w0_sb = pool_w.tile([128, NC_, 128], BF16, tag="w0")
        nc.sync.dma_start(
            w0_sb[:],
            wqk_d.ap()[0:C, 0:128].rearrange("(c p) f -> p c f", p=128),
        )
        bqT_sb = pool_c.tile([128, NC_], F32, tag="bqT")
        nc.sync.dma_start(bqT_sb[:], bqT_d.ap()[:])
        masks = pool_c.tile([128, 4, 512], BF16, tag="mk")
        nc.sync.dma_start(masks[:], mk_d.ap().rearrange("p (r i) -> p r i", r=4))

        # warmup matmuls: keep the PE busy (HAM warm) while xT streams in
        wu_ps = ps_a.tile([128, 512], F32, tag="mm", name="warm")
        for i in range(48):
            nc.tensor.matmul(
                wu_ps[:], ones[:, 0:128], ones[:], start=(i == 0), stop=(i == 47)
            )

        # xT split per channel tile: QKV matmuls depend only on their slice
        xT = pool_x.tile([128, NC_, T], BF16)
        xT_ap = xT_d.ap().rearrange("(c p) t -> p c t", p=128)
        for ci in range(NC_):
            nc.sync.dma_start(xT[:, ci], xT_ap[:, ci])

        wv_sb = pool_wv.tile([128, NC_, CS], BF16)
        nc.sync.dma_start(
            wv_sb[:], wv_d.ap()[0:C].rearrange("(c p) f -> p c f", p=128)
        )
        bv_sb = pool_wv.tile([1, CS], BF16)
        nc.sync.dma_start(bv_sb[:], wv_d.ap()[C : C + 1])

        # remaining qk weight m-tiles (cols 128..1024) and wp, prefetched
        wr_sb = pool_w.tile([128, NC_, 7 * 128], BF16, tag="wr")
        nc.sync.dma_start(
            wr_sb[:],
            wqk_d.ap()[0:C, 128 : 8 * 128].rearrange("(c p) f -> p c f", p=128),
        )
        wp_sb = pool_wp.tile([128, CS // 128, C], BF16)
        nc.sync.dma_start(
            wp_sb[:], wp_d.ap().rearrange("(c p) f -> p c f", p=128)
        )

        # qkT[p, m, t]: feature-major q|k activations, feature f = m*128+p
        qkT = pool_qk.tile([128, 2 * CS // 128, T], BF16)
        # v_ext[p, tb, h*65+d]; col h*65+64 holds ones (the denominator trick)
        vext = pool_ve.tile([128, TB, HPC * 65], BF16)
        nc.sync.dma_start(
            vext[:].rearrange("p tb (h s) -> p tb h s", s=65)[:, :, :, 64],
            on_d.ap()[:, 0 : TB * HPC].rearrange("p (tb h) -> p tb h", h=HPC),
        )
        yT = pool_y.tile([128, CS // 128, T], BF16)

        # ---------------- QKV projection ----------------
        def qk_group(m, tch, on_act):
            w = w0_sb[:, :, :] if m == 0 else wr_sb[:, :, ds((m - 1) * 128, 128)]
            ps = ps_a.tile([128, 512], F32, tag="mm", name="qkps")
            for ci in range(NC_):
                nc.tensor.matmul(
                    ps[:], w[:, ci], xT[:, ci, ts(tch, 512)],
                    start=(ci == 0), stop=(ci == NC_ - 1),
                )
            if on_act:
                nc.scalar.activation(
                    qkT[:, m, ts(tch, 512)], ps[:], IDENT,
                    bias=bqT_sb[:, m : m + 1],
                )
            else:
                nc.vector.tensor_scalar(
                    qkT[:, m, ts(tch, 512)], ps[:],
                    bqT_sb[:, m : m + 1], None, ADD,
                )

        def v_tb(tb):
            ps = ps_a.tile([128, 512], F32, tag="mm", name="vps")
            for ci in range(NC_):
                nc.tensor.matmul(
                    ps[:], xT[:, ci, ts(tb, 128)], wv_sb[:, ci],
                    start=(ci == 0), stop=False,
                )
            nc.tensor.matmul(
                ps[:], ones[0:1, 0:128], bv_sb[:], start=False, stop=True
            )
            nc.vector.tensor_copy(
                out=vext[:, tb].rearrange("p (h s) -> p h s", s=65)[:, :, 0:64],
                in_=ps[:].rearrange("p (h d) -> p h d", d=64),
            )

        # ---------------- attention for one head pair ----------------
        def attn_pair(p, fillers=()):
            fillers = list(fillers)
            nfill = len(fillers)
            npos = 2 * TCH * (TCH + 1) // 2  # 20 jp positions
            pos = 0
            for I in range(TCH):
                nj = 4 * I + 4
                psy = [
                    ps_y.tile([65, 512], F32, tag="psy", name=f"psy{hb_}")
                    for hb_ in range(2)
                ]
                for jp in range(nj // 2):
                    pss = [
                        ps_s.tile([128, 1024], F32, tag="sc", name=f"pss{hb_}")
                        for hb_ in range(2)
                    ]
                    r0 = 2 * jp - 4 * I
                    for hb in range(2):
                        base = hb * 64
                        for jj in range(2):
                            j = 2 * jp + jj
                            o = max(j - 4 * I, 0) * 128
                            nc.tensor.matmul(
                                pss[hb][:, ds(jj * 512 + o, 512 - o)],
                                qkT[base : base + 64, 4 + p, ts(j, 128)],
                                qkT[base : base + 64, p, ds(I * 512 + o, 512 - o)],
                            )
                    st = max(r0, 0) * 128
                    P = [
                        pool_p.tile([128, 1024], BF16, tag="P", name=f"P{hb_}")
                        for hb_ in range(2)
                    ]
                    for hb in range(2):
                        nc.scalar.activation(
                            P[hb][:, ds(st, 1024 - st)],
                            pss[hb][:, ds(st, 1024 - st)], EXP,
                            scale=float(SCALE),
                        )
                    for jj in range(2):
                        r = 2 * jp + jj - 4 * I
                        if r >= 0:
                            o = 128 * r
                            for hb in range(2):
                                nc.vector.tensor_tensor(
                                    P[hb][:, ds(jj * 512 + o, 512 - o)],
                                    P[hb][:, ds(jj * 512 + o, 512 - o)],
                                    masks[:, r, ds(o, 512 - o)],
                                    MULT,
                                )
                    for jj in range(2):
                        j = 2 * jp + jj
                        o = max(j - 4 * I, 0) * 128
                        for hb in range(2):
                            h = 2 * p + hb
                            nc.tensor.matmul(
                                psy[hb][:, ds(o, 512 - o)],
                                vext[:, j, ds(h * 65, 65)],
                                P[hb][:, ds(jj * 512 + o, 512 - o)],
                                start=(j == 0),
                                stop=(j == nj - 1),
                            )
                    # filler: independent matmul group to hide exp latency
                    pos += 1
                    while fillers and len(fillers) > nfill * (npos - pos) // npos:
                        fillers.pop(0)()
                # per-I epilogue: evacuate psy, reciprocal of the ones-row,
                # gpsimd-broadcast, normalize, DMA into yT.  Overlaps the
                # next chunk's attention; unblocks proj per-tch early.
                den = pool_r.tile([8, 128], F32R, tag="den")
                psbs = []
                for hb in range(2):
                    psb = pool_tmp.tile([65, 512], F32R, tag="psb")
                    nc.vector.tensor_copy(out=psb[:], in_=psy[hb][:])
                    nc.sync.dma_start(den[hb * 4 : hb * 4 + 4, :], psb[64:65, :])
                    psbs.append(psb)
                rec = pool_r.tile([8, 128], F32R, tag="rec")
                nc.vector.reciprocal(rec[:], den[:])
                for hb in range(2):
                    rec0 = pool_r.tile([1, 512], F32R, tag="rec0")
                    nc.sync.dma_start(rec0[:], rec[hb * 4 : hb * 4 + 4, :])
                    bc = pool_r.tile([64, 512], F32R, tag="bc")
                    nc.gpsimd.partition_broadcast(bc[:], rec0[:])
                    yn = pool_tmp.tile([64, 512], BF16, tag="yn")
                    nc.vector.tensor_tensor(yn[:], psbs[hb][0:64, :], bc[:], MULT)
                    nc.sync.dma_start(
                        yT[hb * 64 : hb * 64 + 64, p, ds(I * 512, 512)], yn[:]
                    )

        # ---------------- output projection (one token chunk) ----------------
        outT_ap = outT_d.ap().rearrange("(co p) t -> p co t", p=128)

        def proj_group(tch, co):
            ps = ps_a.tile([128, 512], F32, tag="mm", name="projps")
            for cit in range(CS // 128):
                nc.tensor.matmul(
                    ps[:],
                    wp_sb[:, cit, ts(co, 128)],
                    yT[:, cit, ts(tch, 512)],
                    start=(cit == 0),
                    stop=(cit == CS // 128 - 1),
                )
            ot = pool_o.tile([128, 512], F32, tag="out")
            nc.scalar.copy(ot[:], ps[:])
            nc.sync.dma_start(outT_ap[:, co, ts(tch, 512)], ot[:])

        # emission order = scheduling priority: QKV for pair 0 first, then
        # interleave remaining QKV m-tiles with attention pairs so the PE
        # stays fed while ACT grinds through exp; proj per-tch at the end
        # starts as soon as pair 3 finishes each chunk.
        for tch in range(TCH):
            qk_group(0, tch, on_act=True)
        for tch in range(TCH):
            qk_group(4, tch, on_act=True)
        for tb in range(4):
            v_tb(tb)
        f0 = [(lambda tb=tb: v_tb(tb)) for tb in range(4, TB)]
        for m in (1, 5):
            f0 += [(lambda m=m, t=t: qk_group(m, t, False)) for t in range(TCH)]
        attn_pair(0, f0)
        f1 = [(lambda m=m, t=t: qk_group(m, t, False))
              for m in (2, 6) for t in range(TCH)]
        attn_pair(1, f1)
        f2 = [(lambda m=m, t=t: qk_group(m, t, False))
              for m in (3, 7) for t in range(TCH)]
        attn_pair(2, f2)
        attn_pair(3)
        for tch in range(TCH):
            for co in range(C // 128):
                proj_group(tch, co)

    nc.compile()
    return nc


def _masks_host() -> np.ndarray:
    # masks[p, r*512 + i] = 1.0 if i >= r*128 + p else 0.0
    p = np.arange(128)[:, None]
    i = np.arange(512)[None, :]
    out = np.empty((128, 4, 512), dtype=np.float32)
    for r in range(4):
        out[:, r, :] = (i >= r * 128 + p).astype(np.float32)
    return out.reshape(128, 2048)


def kernel(x, w_qkv, b_qkv, w_proj, b_proj):
    x = np.asarray(x, dtype=np.float32)
    w_qkv = np.asarray(w_qkv, dtype=np.float32)
    b_qkv = np.asarray(b_qkv, dtype=np.float32)
    w_proj = np.asarray(w_proj, dtype=np.float32)
    b_proj = np.asarray(b_proj, dtype=np.float32)

    if "nc" not in _CACHE:
        _CACHE["nc"] = _build_program()
    nc = _CACHE["nc"]

    bf = ml_dtypes.bfloat16
    masks = _masks_host().astype(bf)
    ones = np.ones((128, 512), dtype=bf)

    in_maps = []
    for c in range(8):
        b, hg = c // 2, c % 2
        sl = slice(hg * CS, (hg + 1) * CS)
        wq = np.concatenate(
            [w_qkv[:, sl], w_qkv[:, C + hg * CS : C + (hg + 1) * CS]], axis=1
        )
        bq = np.concatenate([b_qkv[sl], b_qkv[C + hg * CS : C + (hg + 1) * CS]])
        wv = w_qkv[:, 2 * C + hg * CS : 2 * C + (hg + 1) * CS]
        bv = b_qkv[2 * C + hg * CS : 2 * C + (hg + 1) * CS]
        in_maps.append({
            "xT": np.ascontiguousarray(x[b].T).astype(bf),
            "wqk": np.ascontiguousarray(wq).astype(bf),
            "bqT": np.ascontiguousarray(bq.reshape(NC_, 128).T).astype(np.float32),
            "wv": np.concatenate([wv, bv[None, :]], axis=0).astype(bf),
            "wp": np.ascontiguousarray(w_proj[hg * CS : (hg + 1) * CS]).astype(bf),
            "masks": masks,
            "ones": ones,
        })

    _CACHE["in_maps"] = in_maps
    res = run_bass_kernel_spmd(nc, in_maps, core_ids=list(range(8)))

    out = np.empty((B, T, C), dtype=np.float32)
    for b in range(B):
        out[b] = res.results[2 * b]["outT"].T
        out[b] += res.results[2 * b + 1]["outT"].T
        out[b] += b_proj
    return out
